# revision 37
# baseline (speedup 1.0000x reference)
"""Causal single-head attention (B=4, T=2048, D=1024, fp32) on 8 TRN2 cores.

Sharding: core c -> (batch b = c//2, parity h = c%2). Each core computes the
output rows for 8 of the 16 query tiles of its batch, chosen to balance the
causal triangle: parity 0 owns tiles {0,3,4,7,8,11,12,15}, parity 1 owns
{1,2,5,6,9,10,13,14}. Tiles are processed in DESCENDING global order, which
makes the set of live query tiles for any key chunk a prefix of the local
tile axis -- the SPMD program is parity-free, with all parity differences
encoded in host-built masks and gathered query stripes.

K/V projections are deduplicated across the core pair: each core projects
only 2 of the 4 key blocks (host feeds it those x^T slabs) and the pair
exchanges blocks with two 2-core AllGathers through DRAM bounce buffers;
both cores then consume K/V for all 4 blocks from the gathered buffer in
global order (parity-uniform addressing).

All matmul inputs are bf16 (fp32 PSUM accumulation): rel error ~5e-3 vs the
2e-2 budget, half the DMA bytes, full PE rate at 128-wide moving dims.

Per-core device program:
  proj own K/V block 0 -> stage -> AllGather r0 (overlapped with Q^T proj)
  Q^T projection; proj own K/V block 1 -> AllGather r1 (overlapped with kb0)
  per key block kb: read gathered K^T/V; per 128-key chunk kc compute
    S^T = K^T.T @ Q^T directly in [k, q] orientation (no PE transposes),
    add the host mask on the last live chunk, exp -> P^T (bf16); per live
    query tile ctx += (P^T).T @ V; one batched ones-matmul chain per kb
    gives the softmax row sums as a [1, n_pos*128] PSUM row.
  ctx and raw row sums are DMA'd out unnormalized; the HOST divides
  (and adds bv: softmax rows sum to 1, so ctx = P@(V+bv) = P@V + bv).
"""

import sys

sys.path.insert(0, "/opt/trn_rl_repo")

import ml_dtypes
import numpy as np

import concourse.mybir as mybir
import concourse.tile as tile
from concourse import bacc
from concourse.bass_utils import run_bass_kernel_spmd

N_CORES = 8
B, T, D = 4, 2048, 1024
P = 128
DC = D // P  # 8 contraction chunks
EC = D // P  # 8 output-feature chunks
KBW = 512  # key-block width
NKB = 4  # key blocks
NOWN = 2  # key blocks projected locally per core
NT = 8  # query tiles per core (of 16 per batch)
NQ = NT * P  # 1024 query rows per core
NEG = -1e30
SCALE = 1.0 / 32.0  # 1/sqrt(D)

# balanced causal split, descending global tile order per parity
QT_DESC = {0: [15, 12, 11, 8, 7, 4, 3, 0], 1: [14, 13, 10, 9, 6, 5, 2, 1]}
# max live query tiles (prefix length) per global key chunk, over both parities
NLM = [8, 8, 7, 7, 6, 6, 5, 5, 4, 4, 3, 3, 2, 2, 1, 1]
NPOS = [8, 6, 4, 2]  # live positions per key block
RSOFF = [0, 8, 14, 18]  # rsout column-block offset per kb (prefix sums)
RSW = 20  # total rsout 128-col blocks

F32 = mybir.dt.float32
BF = mybir.dt.bfloat16
ID = mybir.ActivationFunctionType.Identity
EXP = mybir.ActivationFunctionType.Exp

BF_NP = ml_dtypes.bfloat16
GROUPS = [[0, 1], [2, 3], [4, 5], [6, 7]]
KVW = EC * KBW  # 4096: flat cols of one K^T (or V) block


def _groups(w):
    """Split [0, w) into moving-dim groups of <=512."""
    out = []
    g0 = 0
    while g0 < w:
        out.append((g0, min(512, w - g0)))
        g0 += 512
    return out


def build():
    nc = bacc.Bacc(
        "TRN2", target_bir_lowering=False, debug=False, num_devices=N_CORES
    )
    # all inputs packed host-side so each partition reads contiguous rows:
    # xT/xqT rows ordered (slab, p, dc); weights rows ordered (p, dc)
    xT = nc.dram_tensor(
        "xT", [NOWN * P * DC, KBW], BF, kind="ExternalInput"
    ).ap()
    xqT = nc.dram_tensor(
        "xqT", [(NQ // KBW) * P * DC, KBW], BF, kind="ExternalInput"
    ).ap()
    wqT = nc.dram_tensor("wqT", [P * DC, D], BF, kind="ExternalInput").ap()
    wkT = nc.dram_tensor("wkT", [P * DC, D], BF, kind="ExternalInput").ap()
    wvT = nc.dram_tensor("wvT", [P * DC, D], BF, kind="ExternalInput").ap()
    bq = nc.dram_tensor("bq", [D], F32, kind="ExternalInput").ap()
    cmask = nc.dram_tensor("cmask", [P, 16 * P], F32, kind="ExternalInput").ap()
    out = nc.dram_tensor("out", [NQ, D], F32, kind="ExternalOutput").ap()
    rsum = nc.dram_tensor("rsum", [1, RSW * P], F32, kind="ExternalOutput").ap()

    xT_v = xT.rearrange("(r p dc) c -> r p dc c", r=NOWN, p=P)
    xqT_v = xqT.rearrange("(s p dc) c -> s p dc c", s=NQ // KBW, p=P)
    wq_v = wqT.rearrange("(p dc) e -> p dc e", p=P)
    wk_v = wkT.rearrange("(p dc) e -> p dc e", p=P)
    wv_v = wvT.rearrange("(p dc) e -> p dc e", p=P)

    with tile.TileContext(nc) as tc:
        with (
            tc.tile_pool(name="const", bufs=1) as const,
            tc.tile_pool(name="w", bufs=1) as wpool,
            tc.tile_pool(name="slab", bufs=2) as slab,
            tc.tile_pool(name="big", bufs=1) as big,
            tc.tile_pool(name="kv", bufs=2) as kv,
            tc.tile_pool(name="pt", bufs=2) as ptpool,
            tc.tile_pool(name="dram", bufs=2, space="DRAM") as dram,
            tc.tile_pool(name="mm", bufs=3, space="PSUM") as psmm,
            tc.tile_pool(name="ctx", bufs=1, space="PSUM") as psctx,
        ):
            # DMA issue queues: keep scalar (ACT) free of DMA descriptor
            # work -- it gates every PSUM drain
            dma_rr = [0]
            dma_engs = (nc.sync, nc.gpsimd)

            def dma_rot(dst, src):
                eng = dma_engs[dma_rr[0] % 2]
                dma_rr[0] += 1
                eng.dma_start(out=dst, in_=src)

            # ---- constants / weights / first slabs ----
            # sync: biases, slab r0 (chunked: K proj consumes per-dc), wq, xq
            # gpsimd: wk (chunked, first-use critical), wv, slab r1, masks
            ones = const.tile([P, 1], BF)
            nc.vector.memset(ones, 1.0)

            wq_sb = wpool.tile([P, DC, D], BF, tag="wq")
            wk_sb = wpool.tile([P, DC, D], BF, tag="wk")
            wv_sb = wpool.tile([P, DC, D], BF, tag="wv")

            # first-use-critical loads split [dc0] + [dc1..7]: the first
            # K-proj chain completes as soon as its own inputs land
            # wk (first-use critical, 2MB) split across BOTH queues so it
            # fully lands ~6.5us; dc0 pieces first so chain 1 starts early
            sl_own = [None, None]
            sl_own[0] = slab.tile([P, DC, KBW], BF, tag="slab", name="slo0")
            nc.sync.dma_start(out=sl_own[0][:, 0, :], in_=xT_v[0, :, 0, :])
            nc.gpsimd.dma_start(out=wk_sb[:, 0, :], in_=wk_v[:, 0, :])
            nc.gpsimd.dma_start(out=wk_sb[:, 1:4, :], in_=wk_v[:, 1:4, :])
            nc.sync.dma_start(out=wk_sb[:, 4:, :], in_=wk_v[:, 4:, :])
            nc.sync.dma_start(out=sl_own[0][:, 1:, :], in_=xT_v[0, :, 1:, :])
            nc.gpsimd.dma_start(out=wv_sb, in_=wv_v[:, :, :])
            bq_sb = const.tile([P, EC], F32)
            nc.sync.dma_start(out=bq_sb, in_=bq.rearrange("(c p) -> p c", p=P))
            nc.sync.dma_start(out=wq_sb, in_=wq_v[:, :, :])
            masks = const.tile([P, 16, P], F32)
            nc.sync.dma_start(out=masks, in_=cmask)  # contiguous 2D rows

            qT_sb = big.tile([P, EC, NQ], BF)  # Q^T, [e, local q desc order]
            rsout = big.tile([1, RSW * P], F32)  # raw per-(kb,pos) row sums
            ctx_acc = [
                big.tile([P, D], F32, tag=f"ctx{j}", name=f"ctx{j}")
                for j in range(NT)
            ]

            def allgather(agin, name):
                agout = dram.tile(
                    [2, P, KVW], BF, tag=f"o{name[:3]}", name=f"o{name}"
                )
                nc.gpsimd.collective_compute(
                    "AllGather",
                    mybir.AluOpType.bypass,
                    replica_groups=GROUPS,
                    ins=[agin.opt()],
                    outs=[agout.opt()],
                )
                return agout

            def proj_kv_own(r):
                """Project own K^T/V block r, AllGather each as soon as
                it is staged (K first -- S consumes it before V)."""
                sl = sl_own[r]
                aginK = dram.tile([P, KVW], BF, tag="aginK", name=f"aginK{r}")
                skT = kv.tile([P, EC, KBW], BF, tag="kT", name=f"skT{r}")
                for ec in range(EC):
                    ps = psmm.tile([P, KBW], F32, tag="mm")
                    for dc in range(DC):
                        nc.tensor.matmul(
                            ps,
                            wk_sb[:, dc, ec * P : (ec + 1) * P],
                            sl[:, dc, :],
                            start=(dc == 0),
                            stop=(dc == DC - 1),
                        )
                    # bk dropped: softmax(q.(k+bk)) == softmax(q.k), the
                    # per-row shift q.bk cancels. Plain DVE cast drain --
                    # DVE is not gated by the collective kernel barrier
                    # that blocks the scalar queue for the first ~32us.
                    nc.vector.tensor_copy(skT[:, ec, :], ps)
                dma_rot(aginK, skT[:, :, :])
                agoutK = allgather(aginK, f"K{r}")
                aginV = dram.tile([P, KVW], BF, tag="aginV", name=f"aginV{r}")
                sv = kv.tile([P, 4, KBW * 2], BF, tag="v", name=f"sv{r}")
                for tcc in range(4):
                    for ev in range(2):
                        ps = psmm.tile([P, KBW], F32, tag="mm")
                        for dc in range(DC):
                            nc.tensor.matmul(
                                ps,
                                sl[:, dc, tcc * P : (tcc + 1) * P],
                                wv_sb[:, dc, ev * KBW : (ev + 1) * KBW],
                                start=(dc == 0),
                                stop=(dc == DC - 1),
                            )
                        nc.vector.tensor_copy(
                            sv[:, tcc, ev * KBW : (ev + 1) * KBW], ps
                        )
                dma_rot(aginV, sv[:, :, :])
                agoutV = allgather(aginV, f"V{r}")
                return agoutK, agoutV

            # round 0: own block -> stage -> AllGather (covers kb0 + kb1)
            agK0, agV0 = proj_kv_own(0)

            # prefetch own slab r1, then round 1 (covers kb2 + kb3)
            sl_own[1] = slab.tile([P, DC, KBW], BF, tag="slab", name="slo1")
            nc.gpsimd.dma_start(out=sl_own[1], in_=xT_v[1])
            agK1, agV1 = proj_kv_own(1)
            agKs = [agK0, agK0, agK1, agK1]
            agVs = [agV0, agV0, agV1, agV1]

            # ---- Q^T projection ----
            for s in range(NQ // KBW):
                sl = slab.tile([P, DC, KBW], BF, tag="slab", name=f"qsl{s}")
                nc.sync.dma_start(out=sl, in_=xqT_v[s])
                for ec in range(EC):
                    ps = psmm.tile([P, KBW], F32, tag="mm")
                    for dc in range(DC):
                        nc.tensor.matmul(
                            ps,
                            wq_sb[:, dc, ec * P : (ec + 1) * P],
                            sl[:, dc, :],
                            start=(dc == 0),
                            stop=(dc == DC - 1),
                        )
                    nc.scalar.activation(
                        out=qT_sb[:, ec, s * KBW : (s + 1) * KBW],
                        in_=ps,
                        func=ID,
                        bias=bq_sb[:, ec : ec + 1],
                    )

            # ---- attention per key block ----
            for kb in range(NKB):
                slot = kb % 2
                kT = kv.tile([P, EC, KBW], BF, tag="kT")
                # readback split across both queues: halves the latency
                # between AllGather completion and first S matmul
                nc.sync.dma_start(
                    out=kT[:, 0:4, :], in_=agKs[kb][slot][:, 0 : KVW // 2]
                )
                nc.gpsimd.dma_start(
                    out=kT[:, 4:, :], in_=agKs[kb][slot][:, KVW // 2 : KVW]
                )
                v = kv.tile([P, 4, 2 * KBW], BF, tag="v")
                nc.sync.dma_start(
                    out=v[:, 0:2, :], in_=agVs[kb][slot][:, 0 : KVW // 2]
                )
                nc.gpsimd.dma_start(
                    out=v[:, 2:, :], in_=agVs[kb][slot][:, KVW // 2 : KVW]
                )

                # S^T per key chunk: [k, q] = K^T.T @ Q^T, mask, exp -> P^T
                npos = NPOS[kb]
                pT = ptpool.tile([P, 4, NQ], BF, tag="pT")
                for kcl in range(4):
                    # zero the dead tail so the row-sum chain can read a
                    # uniform [0, npos*P) width per key chunk
                    nlm = NLM[4 * kb + kcl]
                    if nlm < npos:
                        nc.gpsimd.memset(pT[:, kcl, nlm * P : npos * P], 0.0)
                for kcl in range(4):
                    kc = 4 * kb + kcl
                    wq_cols = NLM[kc] * P
                    m0 = (NLM[kc] - 1) * P  # mask column offset
                    for g0, gw in _groups(wq_cols):
                        ps = psmm.tile([P, KBW], F32, tag="mm")
                        for ec in range(EC):
                            nc.tensor.matmul(
                                ps[:, :gw],
                                kT[:, ec, kcl * P : (kcl + 1) * P],
                                qT_sb[:, ec, g0 : g0 + gw],
                                start=(ec == 0),
                                stop=(ec == EC - 1),
                            )
                        if g0 <= m0 < g0 + gw:
                            nc.vector.tensor_add(
                                ps[:, m0 - g0 : m0 - g0 + P],
                                ps[:, m0 - g0 : m0 - g0 + P],
                                masks[:, kc, :],
                            )
                        nc.scalar.activation(
                            out=pT[:, kcl, g0 : g0 + gw],
                            in_=ps[:, :gw],
                            func=EXP,
                            scale=SCALE,
                        )

                # batched row sums for this kb: rs[1, q] += ones.T @ P^T,
                # one chain per <=512-col group over the 4 key chunks
                for g0, gw in _groups(npos * P):
                    rs = psctx.tile([1, KBW], F32, tag="rs", bufs=1, name="rs")
                    for kcl in range(4):
                        nc.tensor.matmul(
                            rs[:, :gw],
                            ones,
                            pT[:, kcl, g0 : g0 + gw],
                            start=(kcl == 0),
                            stop=(kcl == 3),
                        )
                    nc.vector.tensor_copy(
                        rsout[
                            :, (RSOFF[kb] * P + g0) : (RSOFF[kb] * P + g0 + gw)
                        ],
                        rs[:, :gw],
                    )

                # AV: ctx[pos] += P^T.T @ V. Descending pos -> fewest key
                # chunks first -> AV starts before the last exp lands.
                for pos in reversed(range(npos)):
                    kcls = [k for k in range(4) if NLM[4 * kb + k] > pos]
                    n = len(kcls)
                    c0 = psctx.tile([P, KBW], F32, tag="c0", bufs=2, name="c0")
                    c1 = psctx.tile([P, KBW], F32, tag="c1", bufs=2, name="c1")
                    for i, kcl in enumerate(kcls):
                        nc.tensor.matmul(
                            c0,
                            pT[:, kcl, pos * P : (pos + 1) * P],
                            v[:, kcl, 0:KBW],
                            start=(i == 0),
                            stop=(i == n - 1),
                        )
                    for i, kcl in enumerate(kcls):
                        nc.tensor.matmul(
                            c1,
                            pT[:, kcl, pos * P : (pos + 1) * P],
                            v[:, kcl, KBW : 2 * KBW],
                            start=(i == 0),
                            stop=(i == n - 1),
                        )
                    if kb == 0:
                        nc.vector.tensor_copy(ctx_acc[pos][:, 0:KBW], c0)
                        nc.vector.tensor_copy(ctx_acc[pos][:, KBW : 2 * KBW], c1)
                    else:
                        nc.vector.tensor_add(
                            ctx_acc[pos][:, 0:KBW], ctx_acc[pos][:, 0:KBW], c0
                        )
                        nc.vector.tensor_add(
                            ctx_acc[pos][:, KBW : 2 * KBW],
                            ctx_acc[pos][:, KBW : 2 * KBW],
                            c1,
                        )
                    # positions whose causal range ends here: store raw ctx,
                    # one 512-col half per queue so each half streams as
                    # soon as its DVE accumulate lands
                    if pos >= (NPOS[kb + 1] if kb + 1 < NKB else 0):
                        nc.sync.dma_start(
                            out=out[pos * P : (pos + 1) * P, 0:KBW],
                            in_=ctx_acc[pos][:, 0:KBW],
                        )
                        nc.gpsimd.dma_start(
                            out=out[pos * P : (pos + 1) * P, KBW : 2 * KBW],
                            in_=ctx_acc[pos][:, KBW : 2 * KBW],
                        )

            nc.sync.dma_start(out=rsum, in_=rsout)

    nc.compile()
    return nc


_cache = {}


def _get_nc():
    if "nc" not in _cache:
        _cache["nc"] = build()
    return _cache["nc"]


def _host_masks(h: int) -> np.ndarray:
    """16 [P,P] mask blocks in S^T orientation [k_in, q_in], f32 flat [P,16*P].

    Block kc applies to the LAST live chunk (position NLM[kc]-1): triangular
    when that position's tile is the diagonal, all-zero when it's fully live
    (the other parity owns the diagonal), all-NEG when it's dead padding.
    """
    qt = QT_DESC[h]
    m = np.zeros((P, 16, P), dtype=np.float32)
    ki = np.arange(P)[:, None]
    qi = np.arange(P)[None, :]
    for kc in range(16):
        qt_last = qt[NLM[kc] - 1]
        if qt_last == kc:
            m[:, kc, :] = np.where(ki <= qi, 0.0, NEG)
        elif qt_last < kc:
            m[:, kc, :] = NEG
    return m.reshape(P, 16 * P)


def run(inputs, trace: bool = False):
    """Returns (output [B,T,D] fp32, BassKernelResults)."""
    nc = _get_nc()
    x = np.asarray(inputs["x"], dtype=np.float32)
    bq = np.asarray(inputs["bq"], dtype=np.float32)
    bv = np.asarray(inputs["bv"], dtype=np.float32)
    # bk is dropped: it shifts every logit of a softmax row by the same
    # constant q.bk, which softmax cancels exactly.

    def pack_w(w):  # [D, D] -> rows (p, dc), contiguous per partition
        wT = np.asarray(w, dtype=np.float32).T.astype(BF_NP)
        return np.ascontiguousarray(
            wT.reshape(DC, P, D).transpose(1, 0, 2)
        ).reshape(P * DC, D)

    wqT = pack_w(inputs["Wq"])
    wkT = pack_w(inputs["Wk"])
    wvT = pack_w(inputs["Wv"])
    xT = np.transpose(x, (0, 2, 1)).astype(BF_NP)  # [B, D, T] bf16
    # rows ordered (slab, p, dc) so each partition reads contiguous bytes
    xTk = xT.reshape(B, DC, P, NKB, KBW)

    masks = [_host_masks(0), _host_masks(1)]
    in_maps = []
    for c in range(N_CORES):
        b, h = c // 2, c % 2
        own = [h, h + 2]
        xTc = np.ascontiguousarray(
            xTk[b][:, :, own, :].transpose(2, 1, 0, 3)
        ).reshape(NOWN * P * DC, KBW)
        qt = QT_DESC[h]
        qcols = np.concatenate([np.arange(t * P, (t + 1) * P) for t in qt])
        xq = xT[b][:, qcols]  # [D, NQ] bf16
        xqc = np.ascontiguousarray(
            xq.reshape(DC, P, NQ // KBW, KBW).transpose(2, 1, 0, 3)
        ).reshape((NQ // KBW) * P * DC, KBW)
        in_maps.append(
            {
                "xT": xTc,
                "xqT": xqc,
                "wqT": wqT,
                "wkT": wkT,
                "wvT": wvT,
                "bq": bq,
                "cmask": masks[h],
            }
        )

    res = run_bass_kernel_spmd(
        nc, in_maps, core_ids=list(range(N_CORES)), trace=trace
    )

    out = np.empty((B, T, D), dtype=np.float32)
    for c in range(N_CORES):
        b, h = c // 2, c % 2
        qt = QT_DESC[h]
        o = res.results[c]["out"]  # [NQ, D] raw ctx
        rs = res.results[c]["rsum"].reshape(RSW, P)  # per-(kb,pos) sums
        denom = np.zeros((NT, P), dtype=np.float64)
        for kb in range(NKB):
            denom[: NPOS[kb]] += rs[RSOFF[kb] : RSOFF[kb] + NPOS[kb]].astype(
                np.float64
            )
        norm = o / denom.reshape(NQ, 1)
        for pos in range(NT):
            t = qt[pos]
            out[b, t * P : (t + 1) * P, :] = norm[pos * P : (pos + 1) * P, :]
    out += bv  # softmax rows sum to 1, so bv folds out of the attention
    return out, res


def kernel(**inputs) -> np.ndarray:
    out, _ = run(inputs)
    return out


# revision 39
# speedup vs baseline: 1.0262x; 1.0262x over previous
"""Causal single-head attention (B=4, T=2048, D=1024, fp32) on 8 TRN2 cores.

Sharding: core c -> (batch b = c//2, parity h = c%2). Each core computes the
output rows for 8 of the 16 query tiles of its batch, chosen to balance the
causal triangle: parity 0 owns tiles {0,3,4,7,8,11,12,15}, parity 1 owns
{1,2,5,6,9,10,13,14}. Tiles are processed in DESCENDING global order, which
makes the set of live query tiles for any key chunk a prefix of the local
tile axis -- the SPMD program is parity-free, with all parity differences
encoded in host-built masks and gathered query stripes.

K/V projections are deduplicated across the core pair: each core projects
only 2 of the 4 key blocks (host feeds it those x^T slabs) and the pair
exchanges blocks with two 2-core AllGathers through DRAM bounce buffers;
both cores then consume K/V for all 4 blocks from the gathered buffer in
global order (parity-uniform addressing).

All matmul inputs are bf16 (fp32 PSUM accumulation): rel error ~5e-3 vs the
2e-2 budget, half the DMA bytes, full PE rate at 128-wide moving dims.

Per-core device program:
  proj own K/V block 0 -> stage -> AllGather r0 (overlapped with Q^T proj)
  Q^T projection; proj own K/V block 1 -> AllGather r1 (overlapped with kb0)
  per key block kb: read gathered K^T/V; per 128-key chunk kc compute
    S^T = K^T.T @ Q^T directly in [k, q] orientation (no PE transposes),
    add the host mask on the last live chunk, exp -> P^T (bf16); per live
    query tile ctx += (P^T).T @ V; one batched ones-matmul chain per kb
    gives the softmax row sums as a [1, n_pos*128] PSUM row.
  ctx and raw row sums are DMA'd out unnormalized; the HOST divides
  (and adds bv: softmax rows sum to 1, so ctx = P@(V+bv) = P@V + bv).
"""

import sys

sys.path.insert(0, "/opt/trn_rl_repo")

import ml_dtypes
import numpy as np

import concourse.mybir as mybir
import concourse.tile as tile
from concourse import bacc
from concourse.bass_utils import run_bass_kernel_spmd

N_CORES = 8
B, T, D = 4, 2048, 1024
P = 128
DC = D // P  # 8 contraction chunks
EC = D // P  # 8 output-feature chunks
KBW = 512  # key-block width
NKB = 4  # key blocks
NOWN = 2  # key blocks projected locally per core
NT = 8  # query tiles per core (of 16 per batch)
NQ = NT * P  # 1024 query rows per core
NEG = -1e30
SCALE = 1.0 / 32.0  # 1/sqrt(D)

# balanced causal split, descending global tile order per parity
QT_DESC = {0: [15, 12, 11, 8, 7, 4, 3, 0], 1: [14, 13, 10, 9, 6, 5, 2, 1]}
# max live query tiles (prefix length) per global key chunk, over both parities
NLM = [8, 8, 7, 7, 6, 6, 5, 5, 4, 4, 3, 3, 2, 2, 1, 1]
NPOS = [8, 6, 4, 2]  # live positions per key block
RSOFF = [0, 8, 14, 18]  # rsout column-block offset per kb (prefix sums)
RSW = 20  # total rsout 128-col blocks

F32 = mybir.dt.float32
BF = mybir.dt.bfloat16
ID = mybir.ActivationFunctionType.Identity
EXP = mybir.ActivationFunctionType.Exp

BF_NP = ml_dtypes.bfloat16
GROUPS = [[0, 1], [2, 3], [4, 5], [6, 7]]
KVW = EC * KBW  # 4096: flat cols of one K^T (or V) block


def _groups(w):
    """Split [0, w) into moving-dim groups of <=512."""
    out = []
    g0 = 0
    while g0 < w:
        out.append((g0, min(512, w - g0)))
        g0 += 512
    return out


def build():
    nc = bacc.Bacc(
        "TRN2", target_bir_lowering=False, debug=False, num_devices=N_CORES
    )
    # all inputs packed host-side so each partition reads contiguous rows:
    # xT/xqT rows ordered (slab, p, dc); weights rows ordered (p, dc)
    xT = nc.dram_tensor(
        "xT", [NOWN * P * DC, KBW], BF, kind="ExternalInput"
    ).ap()
    xqT = nc.dram_tensor(
        "xqT", [(NQ // KBW) * P * DC, KBW], BF, kind="ExternalInput"
    ).ap()
    wqT = nc.dram_tensor("wqT", [P * DC, D], BF, kind="ExternalInput").ap()
    wkT = nc.dram_tensor("wkT", [P * DC, D], BF, kind="ExternalInput").ap()
    wvT = nc.dram_tensor("wvT", [P * DC, D], BF, kind="ExternalInput").ap()
    bq = nc.dram_tensor("bq", [D], F32, kind="ExternalInput").ap()
    cmask = nc.dram_tensor("cmask", [P, 16 * P], F32, kind="ExternalInput").ap()
    out = nc.dram_tensor("out", [NQ, D], F32, kind="ExternalOutput").ap()
    rsum = nc.dram_tensor("rsum", [1, RSW * P], F32, kind="ExternalOutput").ap()

    xT_v = xT.rearrange("(r p dc) c -> r p dc c", r=NOWN, p=P)
    xqT_v = xqT.rearrange("(s p dc) c -> s p dc c", s=NQ // KBW, p=P)
    wq_v = wqT.rearrange("(p dc) e -> p dc e", p=P)
    wk_v = wkT.rearrange("(p dc) e -> p dc e", p=P)
    wv_v = wvT.rearrange("(p dc) e -> p dc e", p=P)

    with tile.TileContext(nc) as tc:
        with (
            tc.tile_pool(name="const", bufs=1) as const,
            tc.tile_pool(name="w", bufs=1) as wpool,
            tc.tile_pool(name="slab", bufs=2) as slab,
            tc.tile_pool(name="big", bufs=1) as big,
            tc.tile_pool(name="kv", bufs=2) as kv,
            tc.tile_pool(name="pt", bufs=2) as ptpool,
            tc.tile_pool(name="dram", bufs=2, space="DRAM") as dram,
            tc.tile_pool(name="mm", bufs=3, space="PSUM") as psmm,
            tc.tile_pool(name="ctx", bufs=1, space="PSUM") as psctx,
        ):
            # DMA issue queues: keep scalar (ACT) free of DMA descriptor
            # work -- it gates every PSUM drain
            dma_rr = [0]
            dma_engs = (nc.sync, nc.gpsimd)

            def dma_rot(dst, src):
                eng = dma_engs[dma_rr[0] % 2]
                dma_rr[0] += 1
                eng.dma_start(out=dst, in_=src)

            # ---- constants / weights / first slabs ----
            # sync: biases, slab r0 (chunked: K proj consumes per-dc), wq, xq
            # gpsimd: wk (chunked, first-use critical), wv, slab r1, masks
            ones = const.tile([P, 1], BF)
            nc.vector.memset(ones, 1.0)

            wq_sb = wpool.tile([P, DC, D], BF, tag="wq")
            wk_sb = wpool.tile([P, DC, D], BF, tag="wk")
            wv_sb = wpool.tile([P, DC, D], BF, tag="wv")

            # first-use-critical loads split [dc0] + [dc1..7]: the first
            # K-proj chain completes as soon as its own inputs land
            sl_own = [None, None]
            sl_own[0] = slab.tile([P, DC, KBW], BF, tag="slab", name="slo0")
            nc.sync.dma_start(out=sl_own[0][:, 0, :], in_=xT_v[0, :, 0, :])
            nc.gpsimd.dma_start(out=wk_sb[:, 0, :], in_=wk_v[:, 0, :])
            nc.sync.dma_start(out=sl_own[0][:, 1:, :], in_=xT_v[0, :, 1:, :])
            nc.gpsimd.dma_start(out=wk_sb[:, 1:, :], in_=wk_v[:, 1:, :])
            bq_sb = const.tile([P, EC], F32)
            nc.sync.dma_start(out=bq_sb, in_=bq.rearrange("(c p) -> p c", p=P))
            nc.sync.dma_start(out=wq_sb, in_=wq_v[:, :, :])
            nc.gpsimd.dma_start(out=wv_sb, in_=wv_v[:, :, :])
            masks = const.tile([P, 16, P], F32)
            nc.sync.dma_start(out=masks, in_=cmask)  # contiguous 2D rows

            qT_sb = big.tile([P, EC, NQ], BF)  # Q^T, [e, local q desc order]
            rsout = big.tile([1, RSW * P], F32)  # raw per-(kb,pos) row sums
            ctx_acc = [
                big.tile([P, D], F32, tag=f"ctx{j}", name=f"ctx{j}")
                for j in range(NT)
            ]

            def allgather(agin, name):
                agout = dram.tile(
                    [2, P, KVW], BF, tag=f"o{name[:3]}", name=f"o{name}"
                )
                nc.gpsimd.collective_compute(
                    "AllGather",
                    mybir.AluOpType.bypass,
                    replica_groups=GROUPS,
                    ins=[agin.opt()],
                    outs=[agout.opt()],
                )
                return agout

            def proj_kv_own(r):
                """Project own K^T/V block r, AllGather each as soon as
                it is staged (K first -- S consumes it before V)."""
                sl = sl_own[r]
                aginK = dram.tile([P, KVW], BF, tag="aginK", name=f"aginK{r}")
                skT = kv.tile([P, EC, KBW], BF, tag="kT", name=f"skT{r}")
                for ec in range(EC):
                    ps = psmm.tile([P, KBW], F32, tag="mm")
                    for dc in range(DC):
                        nc.tensor.matmul(
                            ps,
                            wk_sb[:, dc, ec * P : (ec + 1) * P],
                            sl[:, dc, :],
                            start=(dc == 0),
                            stop=(dc == DC - 1),
                        )
                    # bk dropped: softmax(q.(k+bk)) == softmax(q.k), the
                    # per-row shift q.bk cancels. Plain DVE cast drain --
                    # DVE is not gated by the collective kernel barrier
                    # that blocks the scalar queue for the first ~32us.
                    nc.vector.tensor_copy(skT[:, ec, :], ps)
                dma_rot(aginK, skT[:, :, :])
                agoutK = allgather(aginK, f"K{r}")
                aginV = dram.tile([P, KVW], BF, tag="aginV", name=f"aginV{r}")
                sv = kv.tile([P, 4, KBW * 2], BF, tag="v", name=f"sv{r}")
                for tcc in range(4):
                    for ev in range(2):
                        ps = psmm.tile([P, KBW], F32, tag="mm")
                        for dc in range(DC):
                            nc.tensor.matmul(
                                ps,
                                sl[:, dc, tcc * P : (tcc + 1) * P],
                                wv_sb[:, dc, ev * KBW : (ev + 1) * KBW],
                                start=(dc == 0),
                                stop=(dc == DC - 1),
                            )
                        nc.vector.tensor_copy(
                            sv[:, tcc, ev * KBW : (ev + 1) * KBW], ps
                        )
                dma_rot(aginV, sv[:, :, :])
                agoutV = allgather(aginV, f"V{r}")
                return agoutK, agoutV

            # round 0: own block -> stage -> AllGather (covers kb0 + kb1)
            agK0, agV0 = proj_kv_own(0)

            # prefetch own slab r1, then round 1 (covers kb2 + kb3)
            sl_own[1] = slab.tile([P, DC, KBW], BF, tag="slab", name="slo1")
            nc.gpsimd.dma_start(out=sl_own[1], in_=xT_v[1])
            agK1, agV1 = proj_kv_own(1)
            agKs = [agK0, agK0, agK1, agK1]
            agVs = [agV0, agV0, agV1, agV1]

            # ---- Q^T projection ----
            for s in range(NQ // KBW):
                sl = slab.tile([P, DC, KBW], BF, tag="slab", name=f"qsl{s}")
                nc.sync.dma_start(out=sl, in_=xqT_v[s])
                for ec in range(EC):
                    ps = psmm.tile([P, KBW], F32, tag="mm")
                    for dc in range(DC):
                        nc.tensor.matmul(
                            ps,
                            wq_sb[:, dc, ec * P : (ec + 1) * P],
                            sl[:, dc, :],
                            start=(dc == 0),
                            stop=(dc == DC - 1),
                        )
                    nc.scalar.activation(
                        out=qT_sb[:, ec, s * KBW : (s + 1) * KBW],
                        in_=ps,
                        func=ID,
                        bias=bq_sb[:, ec : ec + 1],
                    )

            # ---- attention per key block ----
            for kb in range(NKB):
                slot = kb % 2
                # readback halves on sync+scalar: halves the latency between
                # AllGather completion and first use, without adding issue
                # work on gpsimd (the collective trigger queue)
                kT = kv.tile([P, EC, KBW], BF, tag="kT")
                nc.sync.dma_start(
                    out=kT[:, 0:4, :], in_=agKs[kb][slot][:, 0 : KVW // 2]
                )
                nc.scalar.dma_start(
                    out=kT[:, 4:, :], in_=agKs[kb][slot][:, KVW // 2 : KVW]
                )
                v = kv.tile([P, 4, 2 * KBW], BF, tag="v")
                nc.sync.dma_start(
                    out=v[:, 0:2, :], in_=agVs[kb][slot][:, 0 : KVW // 2]
                )
                nc.scalar.dma_start(
                    out=v[:, 2:, :], in_=agVs[kb][slot][:, KVW // 2 : KVW]
                )

                # S^T per key chunk: [k, q] = K^T.T @ Q^T, mask, exp -> P^T
                npos = NPOS[kb]
                pT = ptpool.tile([P, 4, NQ], BF, tag="pT")
                for kcl in range(4):
                    # zero the dead tail so the row-sum chain can read a
                    # uniform [0, npos*P) width per key chunk
                    nlm = NLM[4 * kb + kcl]
                    if nlm < npos:
                        nc.gpsimd.memset(pT[:, kcl, nlm * P : npos * P], 0.0)
                for kcl in range(4):
                    kc = 4 * kb + kcl
                    wq_cols = NLM[kc] * P
                    m0 = (NLM[kc] - 1) * P  # mask column offset
                    for g0, gw in _groups(wq_cols):
                        ps = psmm.tile([P, KBW], F32, tag="mm")
                        for ec in range(EC):
                            nc.tensor.matmul(
                                ps[:, :gw],
                                kT[:, ec, kcl * P : (kcl + 1) * P],
                                qT_sb[:, ec, g0 : g0 + gw],
                                start=(ec == 0),
                                stop=(ec == EC - 1),
                            )
                        if g0 <= m0 < g0 + gw:
                            nc.vector.tensor_add(
                                ps[:, m0 - g0 : m0 - g0 + P],
                                ps[:, m0 - g0 : m0 - g0 + P],
                                masks[:, kc, :],
                            )
                        nc.scalar.activation(
                            out=pT[:, kcl, g0 : g0 + gw],
                            in_=ps[:, :gw],
                            func=EXP,
                            scale=SCALE,
                        )

                # batched row sums for this kb: rs[1, q] += ones.T @ P^T,
                # one chain per <=512-col group over the 4 key chunks
                for g0, gw in _groups(npos * P):
                    rs = psctx.tile([1, KBW], F32, tag="rs", bufs=1, name="rs")
                    for kcl in range(4):
                        nc.tensor.matmul(
                            rs[:, :gw],
                            ones,
                            pT[:, kcl, g0 : g0 + gw],
                            start=(kcl == 0),
                            stop=(kcl == 3),
                        )
                    nc.vector.tensor_copy(
                        rsout[
                            :, (RSOFF[kb] * P + g0) : (RSOFF[kb] * P + g0 + gw)
                        ],
                        rs[:, :gw],
                    )

                # AV: ctx[pos] += P^T.T @ V. Descending pos -> fewest key
                # chunks first -> AV starts before the last exp lands.
                for pos in reversed(range(npos)):
                    kcls = [k for k in range(4) if NLM[4 * kb + k] > pos]
                    n = len(kcls)
                    c0 = psctx.tile([P, KBW], F32, tag="c0", bufs=2, name="c0")
                    c1 = psctx.tile([P, KBW], F32, tag="c1", bufs=2, name="c1")
                    for i, kcl in enumerate(kcls):
                        nc.tensor.matmul(
                            c0,
                            pT[:, kcl, pos * P : (pos + 1) * P],
                            v[:, kcl, 0:KBW],
                            start=(i == 0),
                            stop=(i == n - 1),
                        )
                    for i, kcl in enumerate(kcls):
                        nc.tensor.matmul(
                            c1,
                            pT[:, kcl, pos * P : (pos + 1) * P],
                            v[:, kcl, KBW : 2 * KBW],
                            start=(i == 0),
                            stop=(i == n - 1),
                        )
                    if kb == 0:
                        nc.vector.tensor_copy(ctx_acc[pos][:, 0:KBW], c0)
                        nc.vector.tensor_copy(ctx_acc[pos][:, KBW : 2 * KBW], c1)
                    else:
                        nc.vector.tensor_add(
                            ctx_acc[pos][:, 0:KBW], ctx_acc[pos][:, 0:KBW], c0
                        )
                        nc.vector.tensor_add(
                            ctx_acc[pos][:, KBW : 2 * KBW],
                            ctx_acc[pos][:, KBW : 2 * KBW],
                            c1,
                        )
                    # positions whose causal range ends here: store raw ctx
                    if pos >= (NPOS[kb + 1] if kb + 1 < NKB else 0):
                        nc.sync.dma_start(
                            out=out[pos * P : (pos + 1) * P, :],
                            in_=ctx_acc[pos],
                        )

            nc.sync.dma_start(out=rsum, in_=rsout)

    nc.compile()
    return nc


_cache = {}


def _get_nc():
    if "nc" not in _cache:
        _cache["nc"] = build()
    return _cache["nc"]


def _host_masks(h: int) -> np.ndarray:
    """16 [P,P] mask blocks in S^T orientation [k_in, q_in], f32 flat [P,16*P].

    Block kc applies to the LAST live chunk (position NLM[kc]-1): triangular
    when that position's tile is the diagonal, all-zero when it's fully live
    (the other parity owns the diagonal), all-NEG when it's dead padding.
    """
    qt = QT_DESC[h]
    m = np.zeros((P, 16, P), dtype=np.float32)
    ki = np.arange(P)[:, None]
    qi = np.arange(P)[None, :]
    for kc in range(16):
        qt_last = qt[NLM[kc] - 1]
        if qt_last == kc:
            m[:, kc, :] = np.where(ki <= qi, 0.0, NEG)
        elif qt_last < kc:
            m[:, kc, :] = NEG
    return m.reshape(P, 16 * P)


def run(inputs, trace: bool = False):
    """Returns (output [B,T,D] fp32, BassKernelResults)."""
    nc = _get_nc()
    x = np.asarray(inputs["x"], dtype=np.float32)
    bq = np.asarray(inputs["bq"], dtype=np.float32)
    bv = np.asarray(inputs["bv"], dtype=np.float32)
    # bk is dropped: it shifts every logit of a softmax row by the same
    # constant q.bk, which softmax cancels exactly.

    def pack_w(w):  # [D, D] -> rows (p, dc), contiguous per partition
        wT = np.asarray(w, dtype=np.float32).T.astype(BF_NP)
        return np.ascontiguousarray(
            wT.reshape(DC, P, D).transpose(1, 0, 2)
        ).reshape(P * DC, D)

    wqT = pack_w(inputs["Wq"])
    wkT = pack_w(inputs["Wk"])
    wvT = pack_w(inputs["Wv"])
    xT = np.transpose(x, (0, 2, 1)).astype(BF_NP)  # [B, D, T] bf16
    # rows ordered (slab, p, dc) so each partition reads contiguous bytes
    xTk = xT.reshape(B, DC, P, NKB, KBW)

    masks = [_host_masks(0), _host_masks(1)]
    in_maps = []
    for c in range(N_CORES):
        b, h = c // 2, c % 2
        own = [h, h + 2]
        xTc = np.ascontiguousarray(
            xTk[b][:, :, own, :].transpose(2, 1, 0, 3)
        ).reshape(NOWN * P * DC, KBW)
        qt = QT_DESC[h]
        qcols = np.concatenate([np.arange(t * P, (t + 1) * P) for t in qt])
        xq = xT[b][:, qcols]  # [D, NQ] bf16
        xqc = np.ascontiguousarray(
            xq.reshape(DC, P, NQ // KBW, KBW).transpose(2, 1, 0, 3)
        ).reshape((NQ // KBW) * P * DC, KBW)
        in_maps.append(
            {
                "xT": xTc,
                "xqT": xqc,
                "wqT": wqT,
                "wkT": wkT,
                "wvT": wvT,
                "bq": bq,
                "cmask": masks[h],
            }
        )

    res = run_bass_kernel_spmd(
        nc, in_maps, core_ids=list(range(N_CORES)), trace=trace
    )

    out = np.empty((B, T, D), dtype=np.float32)
    for c in range(N_CORES):
        b, h = c // 2, c % 2
        qt = QT_DESC[h]
        o = res.results[c]["out"]  # [NQ, D] raw ctx
        rs = res.results[c]["rsum"].reshape(RSW, P)  # per-(kb,pos) sums
        denom = np.zeros((NT, P), dtype=np.float64)
        for kb in range(NKB):
            denom[: NPOS[kb]] += rs[RSOFF[kb] : RSOFF[kb] + NPOS[kb]].astype(
                np.float64
            )
        norm = o / denom.reshape(NQ, 1)
        for pos in range(NT):
            t = qt[pos]
            out[b, t * P : (t + 1) * P, :] = norm[pos * P : (pos + 1) * P, :]
    out += bv  # softmax rows sum to 1, so bv folds out of the attention
    return out, res


def kernel(**inputs) -> np.ndarray:
    out, _ = run(inputs)
    return out


# revision 40
# speedup vs baseline: 1.1254x; 1.0967x over previous
"""Causal single-head attention (B=4, T=2048, D=1024, fp32) on 8 TRN2 cores.

Sharding: core c -> (batch b = c//2, parity h = c%2). Each core computes the
output rows for 8 of the 16 query tiles of its batch, chosen to balance the
causal triangle: parity 0 owns tiles {0,3,4,7,8,11,12,15}, parity 1 owns
{1,2,5,6,9,10,13,14}. Tiles are processed in DESCENDING global order, which
makes the set of live query tiles for any key chunk a prefix of the local
tile axis -- the SPMD program is parity-free, with all parity differences
encoded in host-built masks and gathered query stripes.

K/V projections are deduplicated across the core pair: each core projects
only 2 of the 4 key blocks (host feeds it those x^T slabs) and the pair
exchanges blocks with two 2-core AllGathers through DRAM bounce buffers;
both cores then consume K/V for all 4 blocks from the gathered buffer in
global order (parity-uniform addressing).

All matmul inputs are bf16 (fp32 PSUM accumulation): rel error ~5e-3 vs the
2e-2 budget, half the DMA bytes, full PE rate at 128-wide moving dims.

Per-core device program:
  proj own K/V block 0 -> stage -> AllGather r0 (overlapped with Q^T proj)
  Q^T projection; proj own K/V block 1 -> AllGather r1 (overlapped with kb0)
  per key block kb: read gathered K^T/V; per 128-key chunk kc compute
    S^T = K^T.T @ Q^T directly in [k, q] orientation (no PE transposes),
    add the host mask on the last live chunk, exp -> P^T (bf16); per live
    query tile ctx += (P^T).T @ V; one batched ones-matmul chain per kb
    gives the softmax row sums as a [1, n_pos*128] PSUM row.
  ctx and raw row sums are DMA'd out unnormalized; the HOST divides
  (and adds bv: softmax rows sum to 1, so ctx = P@(V+bv) = P@V + bv).
"""

import sys

sys.path.insert(0, "/opt/trn_rl_repo")

import ml_dtypes
import numpy as np

import concourse.mybir as mybir
import concourse.tile as tile
from concourse import bacc
from concourse.bass_utils import run_bass_kernel_spmd

N_CORES = 8
B, T, D = 4, 2048, 1024
P = 128
DC = D // P  # 8 contraction chunks
EC = D // P  # 8 output-feature chunks
KBW = 512  # key-block width
NKB = 4  # key blocks
NOWN = 2  # key blocks projected locally per core
NT = 8  # query tiles per core (of 16 per batch)
NQ = NT * P  # 1024 query rows per core
NEG = -1e30
SCALE = 1.0 / 32.0  # 1/sqrt(D)

# balanced causal split, descending global tile order per parity
QT_DESC = {0: [15, 12, 11, 8, 7, 4, 3, 0], 1: [14, 13, 10, 9, 6, 5, 2, 1]}
# max live query tiles (prefix length) per global key chunk, over both parities
NLM = [8, 8, 7, 7, 6, 6, 5, 5, 4, 4, 3, 3, 2, 2, 1, 1]
NPOS = [8, 6, 4, 2]  # live positions per key block
RSOFF = [0, 8, 14, 18]  # rsout column-block offset per kb (prefix sums)
RSW = 20  # total rsout 128-col blocks

F32 = mybir.dt.float32
BF = mybir.dt.bfloat16
ID = mybir.ActivationFunctionType.Identity
EXP = mybir.ActivationFunctionType.Exp

BF_NP = ml_dtypes.bfloat16
GROUPS = [[0, 1], [2, 3], [4, 5], [6, 7]]
KVW = EC * KBW  # 4096: flat cols of one K^T (or V) block


def _groups(w):
    """Split [0, w) into moving-dim groups of <=512."""
    out = []
    g0 = 0
    while g0 < w:
        out.append((g0, min(512, w - g0)))
        g0 += 512
    return out


def build():
    nc = bacc.Bacc(
        "TRN2", target_bir_lowering=False, debug=False, num_devices=N_CORES
    )
    # all inputs packed host-side so each partition reads contiguous rows:
    # xT/xqT rows ordered (slab, p, dc); weights rows ordered (p, dc)
    xT = nc.dram_tensor(
        "xT", [NOWN * P * DC, KBW], BF, kind="ExternalInput"
    ).ap()
    xqT = nc.dram_tensor(
        "xqT", [(NQ // KBW) * P * DC, KBW], BF, kind="ExternalInput"
    ).ap()
    wqT = nc.dram_tensor("wqT", [P * DC, D], BF, kind="ExternalInput").ap()
    wkT = nc.dram_tensor("wkT", [P * DC, D], BF, kind="ExternalInput").ap()
    wvT = nc.dram_tensor("wvT", [P * DC, D], BF, kind="ExternalInput").ap()
    bq = nc.dram_tensor("bq", [D], F32, kind="ExternalInput").ap()
    cmask = nc.dram_tensor("cmask", [P, 16 * P], F32, kind="ExternalInput").ap()
    out = nc.dram_tensor("out", [NQ, D], F32, kind="ExternalOutput").ap()
    rsum = nc.dram_tensor("rsum", [1, RSW * P], F32, kind="ExternalOutput").ap()

    xT_v = xT.rearrange("(r p dc) c -> r p dc c", r=NOWN, p=P)
    xqT_v = xqT.rearrange("(s p dc) c -> s p dc c", s=NQ // KBW, p=P)
    wq_v = wqT.rearrange("(p dc) e -> p dc e", p=P)
    wk_v = wkT.rearrange("(p dc) e -> p dc e", p=P)
    wv_v = wvT.rearrange("(p dc) e -> p dc e", p=P)

    with tile.TileContext(nc) as tc:
        with (
            tc.tile_pool(name="const", bufs=1) as const,
            tc.tile_pool(name="w", bufs=1) as wpool,
            tc.tile_pool(name="slab", bufs=2) as slab,
            tc.tile_pool(name="big", bufs=1) as big,
            tc.tile_pool(name="kv", bufs=2) as kv,
            tc.tile_pool(name="pt", bufs=2) as ptpool,
            tc.tile_pool(name="dram", bufs=2, space="DRAM") as dram,
            tc.tile_pool(name="mm", bufs=3, space="PSUM") as psmm,
            tc.tile_pool(name="ctx", bufs=1, space="PSUM") as psctx,
        ):
            # DMA issue queues: keep scalar (ACT) free of DMA descriptor
            # work -- it gates every PSUM drain
            dma_rr = [0]
            dma_engs = (nc.sync, nc.gpsimd)

            def dma_rot(dst, src):
                eng = dma_engs[dma_rr[0] % 2]
                dma_rr[0] += 1
                eng.dma_start(out=dst, in_=src)

            # ---- constants / weights / first slabs ----
            # sync: biases, slab r0 (chunked: K proj consumes per-dc), wq, xq
            # gpsimd: wk (chunked, first-use critical), wv, slab r1, masks
            ones = const.tile([P, 1], BF)
            nc.vector.memset(ones, 1.0)

            wq_sb = wpool.tile([P, DC, D], BF, tag="wq")
            wk_sb = wpool.tile([P, DC, D], BF, tag="wk")
            wv_sb = wpool.tile([P, DC, D], BF, tag="wv")

            # first-use-critical loads split [dc0] + [dc1..7]: the first
            # K-proj chain completes as soon as its own inputs land
            sl_own = [None, None]
            sl_own[0] = slab.tile([P, DC, KBW], BF, tag="slab", name="slo0")
            nc.sync.dma_start(out=sl_own[0][:, 0, :], in_=xT_v[0, :, 0, :])
            nc.gpsimd.dma_start(out=wk_sb[:, 0, :], in_=wk_v[:, 0, :])
            nc.sync.dma_start(out=sl_own[0][:, 1:, :], in_=xT_v[0, :, 1:, :])
            nc.gpsimd.dma_start(out=wk_sb[:, 1:, :], in_=wk_v[:, 1:, :])
            bq_sb = const.tile([P, EC], F32)
            nc.sync.dma_start(out=bq_sb, in_=bq.rearrange("(c p) -> p c", p=P))
            nc.sync.dma_start(out=wq_sb, in_=wq_v[:, :, :])
            nc.gpsimd.dma_start(out=wv_sb, in_=wv_v[:, :, :])
            masks = const.tile([P, 16, P], F32)
            nc.sync.dma_start(out=masks, in_=cmask)  # contiguous 2D rows

            qT_sb = big.tile([P, EC, NQ], BF)  # Q^T, [e, local q desc order]
            rsout = big.tile([1, RSW * P], F32)  # raw per-(kb,pos) row sums
            ctx_acc = [
                big.tile([P, D], F32, tag=f"ctx{j}", name=f"ctx{j}")
                for j in range(NT)
            ]

            def allgather(agin, name):
                agout = dram.tile(
                    [2, P, KVW], BF, tag=f"o{name[:3]}", name=f"o{name}"
                )
                nc.gpsimd.collective_compute(
                    "AllGather",
                    mybir.AluOpType.bypass,
                    replica_groups=GROUPS,
                    ins=[agin.opt()],
                    outs=[agout.opt()],
                )
                return agout

            def proj_kv_own(r):
                """Project own K^T/V block r, AllGather each as soon as
                it is staged (K first -- S consumes it before V)."""
                sl = sl_own[r]
                aginK = dram.tile([P, KVW], BF, tag="aginK", name=f"aginK{r}")
                skT = kv.tile([P, EC, KBW], BF, tag="kT", name=f"skT{r}")
                for ec in range(EC):
                    ps = psmm.tile([P, KBW], F32, tag="mm")
                    for dc in range(DC):
                        nc.tensor.matmul(
                            ps,
                            wk_sb[:, dc, ec * P : (ec + 1) * P],
                            sl[:, dc, :],
                            start=(dc == 0),
                            stop=(dc == DC - 1),
                        )
                    # bk dropped: softmax(q.(k+bk)) == softmax(q.k), the
                    # per-row shift q.bk cancels. Plain DVE cast drain --
                    # DVE is not gated by the collective kernel barrier
                    # that blocks the scalar queue for the first ~32us.
                    nc.vector.tensor_copy(skT[:, ec, :], ps)
                dma_rot(aginK, skT[:, :, :])
                agoutK = allgather(aginK, f"K{r}")
                aginV = dram.tile([P, KVW], BF, tag="aginV", name=f"aginV{r}")
                sv = kv.tile([P, 4, KBW * 2], BF, tag="v", name=f"sv{r}")
                for tcc in range(4):
                    for ev in range(2):
                        ps = psmm.tile([P, KBW], F32, tag="mm")
                        for dc in range(DC):
                            nc.tensor.matmul(
                                ps,
                                sl[:, dc, tcc * P : (tcc + 1) * P],
                                wv_sb[:, dc, ev * KBW : (ev + 1) * KBW],
                                start=(dc == 0),
                                stop=(dc == DC - 1),
                            )
                        nc.vector.tensor_copy(
                            sv[:, tcc, ev * KBW : (ev + 1) * KBW], ps
                        )
                dma_rot(aginV, sv[:, :, :])
                agoutV = allgather(aginV, f"V{r}")
                return agoutK, agoutV

            # round 0: own block -> stage -> AllGather (covers kb0 + kb1)
            agK0, agV0 = proj_kv_own(0)

            # prefetch own slab r1, then round 1 (covers kb2 + kb3)
            sl_own[1] = slab.tile([P, DC, KBW], BF, tag="slab", name="slo1")
            nc.gpsimd.dma_start(out=sl_own[1], in_=xT_v[1])
            agK1, agV1 = proj_kv_own(1)
            agKs = [agK0, agK0, agK1, agK1]
            agVs = [agV0, agV0, agV1, agV1]

            # ---- Q^T projection ----
            for s in range(NQ // KBW):
                sl = slab.tile([P, DC, KBW], BF, tag="slab", name=f"qsl{s}")
                nc.sync.dma_start(out=sl, in_=xqT_v[s])
                for ec in range(EC):
                    ps = psmm.tile([P, KBW], F32, tag="mm")
                    for dc in range(DC):
                        nc.tensor.matmul(
                            ps,
                            wq_sb[:, dc, ec * P : (ec + 1) * P],
                            sl[:, dc, :],
                            start=(dc == 0),
                            stop=(dc == DC - 1),
                        )
                    nc.scalar.activation(
                        out=qT_sb[:, ec, s * KBW : (s + 1) * KBW],
                        in_=ps,
                        func=ID,
                        bias=bq_sb[:, ec : ec + 1],
                    )

            # ---- attention per key block ----
            for kb in range(NKB):
                slot = kb % 2
                kT = kv.tile([P, EC, KBW], BF, tag="kT")
                dma_rot(kT[:, :, :], agKs[kb][slot])
                v = kv.tile([P, 4, 2 * KBW], BF, tag="v")
                dma_rot(v[:, :, :], agVs[kb][slot])

                # S^T per key chunk: [k, q] = K^T.T @ Q^T, mask, exp -> P^T
                npos = NPOS[kb]
                pT = ptpool.tile([P, 4, NQ], BF, tag="pT")
                for kcl in range(4):
                    # zero the dead tail so the row-sum chain can read a
                    # uniform [0, npos*P) width per key chunk
                    nlm = NLM[4 * kb + kcl]
                    if nlm < npos:
                        nc.gpsimd.memset(pT[:, kcl, nlm * P : npos * P], 0.0)
                for kcl in range(4):
                    kc = 4 * kb + kcl
                    wq_cols = NLM[kc] * P
                    m0 = (NLM[kc] - 1) * P  # mask column offset
                    for g0, gw in _groups(wq_cols):
                        ps = psmm.tile([P, KBW], F32, tag="mm")
                        for ec in range(EC):
                            nc.tensor.matmul(
                                ps[:, :gw],
                                kT[:, ec, kcl * P : (kcl + 1) * P],
                                qT_sb[:, ec, g0 : g0 + gw],
                                start=(ec == 0),
                                stop=(ec == EC - 1),
                            )
                        if g0 <= m0 < g0 + gw:
                            nc.vector.tensor_add(
                                ps[:, m0 - g0 : m0 - g0 + P],
                                ps[:, m0 - g0 : m0 - g0 + P],
                                masks[:, kc, :],
                            )
                        nc.scalar.activation(
                            out=pT[:, kcl, g0 : g0 + gw],
                            in_=ps[:, :gw],
                            func=EXP,
                            scale=SCALE,
                        )

                # batched row sums for this kb: rs[1, q] += ones.T @ P^T,
                # one chain per <=512-col group over the 4 key chunks
                for g0, gw in _groups(npos * P):
                    rs = psctx.tile([1, KBW], F32, tag="rs", bufs=1, name="rs")
                    for kcl in range(4):
                        nc.tensor.matmul(
                            rs[:, :gw],
                            ones,
                            pT[:, kcl, g0 : g0 + gw],
                            start=(kcl == 0),
                            stop=(kcl == 3),
                        )
                    nc.vector.tensor_copy(
                        rsout[
                            :, (RSOFF[kb] * P + g0) : (RSOFF[kb] * P + g0 + gw)
                        ],
                        rs[:, :gw],
                    )

                # AV: ctx[pos] += P^T.T @ V. Descending pos -> fewest key
                # chunks first -> AV starts before the last exp lands.
                for pos in reversed(range(npos)):
                    kcls = [k for k in range(4) if NLM[4 * kb + k] > pos]
                    n = len(kcls)
                    c0 = psctx.tile([P, KBW], F32, tag="c0", bufs=2, name="c0")
                    c1 = psctx.tile([P, KBW], F32, tag="c1", bufs=2, name="c1")
                    for i, kcl in enumerate(kcls):
                        nc.tensor.matmul(
                            c0,
                            pT[:, kcl, pos * P : (pos + 1) * P],
                            v[:, kcl, 0:KBW],
                            start=(i == 0),
                            stop=(i == n - 1),
                        )
                    for i, kcl in enumerate(kcls):
                        nc.tensor.matmul(
                            c1,
                            pT[:, kcl, pos * P : (pos + 1) * P],
                            v[:, kcl, KBW : 2 * KBW],
                            start=(i == 0),
                            stop=(i == n - 1),
                        )
                    if kb == 0:
                        nc.vector.tensor_copy(ctx_acc[pos][:, 0:KBW], c0)
                        nc.vector.tensor_copy(ctx_acc[pos][:, KBW : 2 * KBW], c1)
                    else:
                        nc.vector.tensor_add(
                            ctx_acc[pos][:, 0:KBW], ctx_acc[pos][:, 0:KBW], c0
                        )
                        nc.vector.tensor_add(
                            ctx_acc[pos][:, KBW : 2 * KBW],
                            ctx_acc[pos][:, KBW : 2 * KBW],
                            c1,
                        )
                    # positions whose causal range ends here: store raw ctx
                    if pos >= (NPOS[kb + 1] if kb + 1 < NKB else 0):
                        nc.sync.dma_start(
                            out=out[pos * P : (pos + 1) * P, :],
                            in_=ctx_acc[pos],
                        )

            nc.sync.dma_start(out=rsum, in_=rsout)

    nc.compile()
    return nc


_cache = {}


def _get_nc():
    if "nc" not in _cache:
        _cache["nc"] = build()
    return _cache["nc"]


def _host_masks(h: int) -> np.ndarray:
    """16 [P,P] mask blocks in S^T orientation [k_in, q_in], f32 flat [P,16*P].

    Block kc applies to the LAST live chunk (position NLM[kc]-1): triangular
    when that position's tile is the diagonal, all-zero when it's fully live
    (the other parity owns the diagonal), all-NEG when it's dead padding.
    """
    qt = QT_DESC[h]
    m = np.zeros((P, 16, P), dtype=np.float32)
    ki = np.arange(P)[:, None]
    qi = np.arange(P)[None, :]
    for kc in range(16):
        qt_last = qt[NLM[kc] - 1]
        if qt_last == kc:
            m[:, kc, :] = np.where(ki <= qi, 0.0, NEG)
        elif qt_last < kc:
            m[:, kc, :] = NEG
    return m.reshape(P, 16 * P)


def run(inputs, trace: bool = False):
    """Returns (output [B,T,D] fp32, BassKernelResults)."""
    nc = _get_nc()
    x = np.asarray(inputs["x"], dtype=np.float32)
    bq = np.asarray(inputs["bq"], dtype=np.float32)
    bv = np.asarray(inputs["bv"], dtype=np.float32)
    # bk is dropped: it shifts every logit of a softmax row by the same
    # constant q.bk, which softmax cancels exactly.

    def pack_w(w):  # [D, D] -> rows (p, dc), contiguous per partition
        wT = np.asarray(w, dtype=np.float32).T.astype(BF_NP)
        return np.ascontiguousarray(
            wT.reshape(DC, P, D).transpose(1, 0, 2)
        ).reshape(P * DC, D)

    wqT = pack_w(inputs["Wq"])
    wkT = pack_w(inputs["Wk"])
    wvT = pack_w(inputs["Wv"])
    xT = np.transpose(x, (0, 2, 1)).astype(BF_NP)  # [B, D, T] bf16
    # rows ordered (slab, p, dc) so each partition reads contiguous bytes
    xTk = xT.reshape(B, DC, P, NKB, KBW)

    masks = [_host_masks(0), _host_masks(1)]
    in_maps = []
    for c in range(N_CORES):
        b, h = c // 2, c % 2
        own = [h, h + 2]
        xTc = np.ascontiguousarray(
            xTk[b][:, :, own, :].transpose(2, 1, 0, 3)
        ).reshape(NOWN * P * DC, KBW)
        qt = QT_DESC[h]
        qcols = np.concatenate([np.arange(t * P, (t + 1) * P) for t in qt])
        xq = xT[b][:, qcols]  # [D, NQ] bf16
        xqc = np.ascontiguousarray(
            xq.reshape(DC, P, NQ // KBW, KBW).transpose(2, 1, 0, 3)
        ).reshape((NQ // KBW) * P * DC, KBW)
        in_maps.append(
            {
                "xT": xTc,
                "xqT": xqc,
                "wqT": wqT,
                "wkT": wkT,
                "wvT": wvT,
                "bq": bq,
                "cmask": masks[h],
            }
        )

    res = run_bass_kernel_spmd(
        nc, in_maps, core_ids=list(range(N_CORES)), trace=trace
    )

    out = np.empty((B, T, D), dtype=np.float32)
    for c in range(N_CORES):
        b, h = c // 2, c % 2
        qt = QT_DESC[h]
        o = res.results[c]["out"]  # [NQ, D] raw ctx
        rs = res.results[c]["rsum"].reshape(RSW, P)  # per-(kb,pos) sums
        denom = np.zeros((NT, P), dtype=np.float64)
        for kb in range(NKB):
            denom[: NPOS[kb]] += rs[RSOFF[kb] : RSOFF[kb] + NPOS[kb]].astype(
                np.float64
            )
        norm = o / denom.reshape(NQ, 1)
        for pos in range(NT):
            t = qt[pos]
            out[b, t * P : (t + 1) * P, :] = norm[pos * P : (pos + 1) * P, :]
    out += bv  # softmax rows sum to 1, so bv folds out of the attention
    return out, res


def kernel(**inputs) -> np.ndarray:
    out, _ = run(inputs)
    return out
